# revision 1
# baseline (speedup 1.0000x reference)
"""Trainium2 Bass kernel v2 for edge-attention GNN message passing.

  q,k,v = x @ {Wq,Wk,Wv}.T  (per-head split)
  alpha[e,h] = sum_d q[dst,h,d]*w[e,h,d]*k[src,h,d] / sqrt(hd) * cutoff[e]
  out = segment_sum(alpha[...,None]*v[src], dst)

Zero-gather design (8 cores, SPMD single program):
  - dst-partitioned: core c owns dst nodes [c*6250, (c+1)*6250).
  - Host packs each core's edges into chunks of <=128 consecutive dst nodes
    and <=1536 edges (12 subchunks of 128 slots), then supplies per-slot
    streams: w (cutoff-folded, slot-major), x[src] and x[dst]
    (feature-major for PE stationary use), plus slot->node pos.
  - Device per subchunk: project k|v = xe.T@[WkT|WvT] and q = xq.T@WqT on
    PE (per-edge projection; gather done on host), alpha via two muls +
    per-head reduce, msgs = alpha*v in place, segment-sum via one-hot
    S.T @ msgs accumulated in PSUM per chunk.
  - No DRAM tables, no dma_gather, single phase.
"""

import numpy as np
import ml_dtypes

import concourse.bacc as bacc
import concourse.bass as bass
import concourse.mybir as mybir
import concourse.tile as tile
from concourse.bass_utils import run_bass_kernel_spmd

F32 = mybir.dt.float32
BF16 = mybir.dt.bfloat16
NBF16 = ml_dtypes.bfloat16

N_NODES = 50000
N_EDGES = 600000
HID = 128
NH = 8
HD = 16
NCORE = 8
NPC = N_NODES // NCORE
P = 128

SUBS = 12                         # subchunks per chunk
SLOTS = SUBS * P                  # 1536 edge slots per chunk
G = 4                             # chunks per group
GCOL = G * SUBS                   # 48 subchunk columns per group
GSLOT = G * SLOTS                 # 6144 slots per group

_cache = {}


def _build_program(ngroup, repeat=1):
    nc = bacc.Bacc("TRN2", target_bir_lowering=False, debug=False,
                   num_devices=NCORE)

    wkv_d = nc.dram_tensor("wkv_d", [HID, 2 * HID], BF16, kind="ExternalInput")
    wq_d = nc.dram_tensor("wq_d", [HID, HID], BF16, kind="ExternalInput")
    iota_d = nc.dram_tensor("iota_d", [P, P], BF16, kind="ExternalInput")
    pos_d = nc.dram_tensor("pos_d", [P, ngroup * GCOL], BF16,
                           kind="ExternalInput")
    we_d = nc.dram_tensor("we_d", [ngroup, P, GCOL, HID], BF16,
                          kind="ExternalInput")
    xs_d = nc.dram_tensor("xs_d", [ngroup, P, GCOL, P], BF16,
                          kind="ExternalInput")
    xq_d = nc.dram_tensor("xq_d", [ngroup, P, GCOL, P], BF16,
                          kind="ExternalInput")
    out_d = nc.dram_tensor("out_d", [ngroup * G * P, HID], F32,
                           kind="ExternalOutput")

    with tile.TileContext(nc) as tc:
        with tc.tile_pool(name="cst", bufs=1) as cst, \
             tc.tile_pool(name="big", bufs=2) as big, \
             tc.tile_pool(name="pps", bufs=3, space="PSUM") as pps, \
             tc.tile_pool(name="qps", bufs=2, space="PSUM") as qps, \
             tc.tile_pool(name="ops", bufs=3, space="PSUM") as ops:
            wkv = cst.tile([HID, 2 * HID], BF16)
            nc.sync.dma_start(wkv[:], wkv_d[:])
            wq = cst.tile([HID, HID], BF16)
            nc.sync.dma_start(wq[:], wq_d[:])
            iota = cst.tile([P, P], BF16)
            nc.sync.dma_start(iota[:], iota_d[:])
            pos = cst.tile([P, ngroup * GCOL], BF16)
            nc.sync.dma_start(pos[:], pos_d[:])

            for g in [g for _ in range(repeat) for g in range(ngroup)]:
                s0 = g * GCOL
                xe_t = big.tile([P, GCOL, P], BF16, tag="xe")
                nc.sync.dma_start(xe_t[:], xs_d[g])
                xq_t = big.tile([P, GCOL, P], BF16, tag="xq")
                nc.sync.dma_start(xq_t[:], xq_d[g])
                w_t = big.tile([P, GCOL, HID], BF16, tag="w")
                nc.sync.dma_start(w_t[:], we_d[g])

                k_t = big.tile([P, GCOL, HID], BF16, tag="k")
                v_t = big.tile([P, GCOL, HID], BF16, tag="v")
                qg_t = big.tile([P, GCOL, HID], BF16, tag="qg")
                # kv: 2 subchunks per PSUM bank; q: 4 per bank. Fused copies.
                for j0 in range(0, GCOL, 2):
                    kvp = pps.tile([P, 2, 2 * HID], F32, tag="kvp")
                    for t in range(2):
                        nc.tensor.matmul(kvp[:, t, :], xe_t[:, j0 + t, :],
                                         wkv[:], start=True, stop=True)
                    nc.any.tensor_copy(
                        out=k_t[:, j0:j0 + 2, :], in_=kvp[:, :, 0:HID])
                    nc.any.tensor_copy(
                        out=v_t[:, j0:j0 + 2, :], in_=kvp[:, :, HID:2 * HID])
                for j0 in range(0, GCOL, 4):
                    qp = qps.tile([P, 4, HID], F32, tag="qp")
                    for t in range(4):
                        nc.tensor.matmul(qp[:, t, :], xq_t[:, j0 + t, :],
                                         wq[:], start=True, stop=True)
                    nc.any.tensor_copy(out=qg_t[:, j0:j0 + 4, :], in_=qp[:])

                S_t = big.tile([P, GCOL, P], BF16, tag="S")
                nc.any.tensor_tensor(
                    out=S_t[:],
                    in0=iota[:].unsqueeze(1).to_broadcast([P, GCOL, P]),
                    in1=pos[:, s0:s0 + GCOL].unsqueeze(2).to_broadcast(
                        [P, GCOL, P]),
                    op=mybir.AluOpType.is_equal)
                # t1 = w*k, then t1 *= q (both in place over w_t)
                nc.any.tensor_tensor(out=w_t[:], in0=w_t[:], in1=k_t[:],
                                     op=mybir.AluOpType.mult)
                nc.vector.tensor_tensor(out=w_t[:], in0=w_t[:], in1=qg_t[:],
                                        op=mybir.AluOpType.mult)
                alpha_t = big.tile([P, GCOL, NH], F32, tag="alpha")
                nc.vector.tensor_reduce(
                    out=alpha_t[:],
                    in_=w_t[:].rearrange("p c (h x) -> p c h x", x=HD),
                    axis=mybir.AxisListType.X, op=mybir.AluOpType.add)
                alpha2_t = big.tile([P, GCOL, NH], BF16, tag="alpha2")
                nc.any.tensor_copy(out=alpha2_t[:], in_=alpha_t[:])
                # msgs = v * alpha (in place on v_t)
                nc.vector.tensor_tensor(
                    out=v_t[:].rearrange("p c (h x) -> p c h x", x=HD),
                    in0=v_t[:].rearrange("p c (h x) -> p c h x", x=HD),
                    in1=alpha2_t[:].unsqueeze(3).to_broadcast(
                        [P, GCOL, NH, HD]),
                    op=mybir.AluOpType.mult)

                out_t = big.tile([P, G, HID], F32, tag="out")
                for c in range(G):
                    acc = ops.tile([P, HID], F32, tag="acc")
                    for m in range(SUBS):
                        j = c * SUBS + m
                        nc.tensor.matmul(acc[:], S_t[:, j, :],
                                         v_t[:, j, :],
                                         start=(m == 0),
                                         stop=(m == SUBS - 1))
                    nc.any.tensor_copy(out=out_t[:, c, :], in_=acc[:])
                nc.sync.dma_start(
                    out_d[g * G * P:(g + 1) * G * P, :].rearrange(
                        "(c p) d -> p c d", p=P),
                    out_t[:])
    nc.compile()
    return nc


def _pack_core(ld, nchunk_target=None):
    """Greedy chunking of one core's dst-sorted edges: <=128 consecutive
    nodes and <=SLOTS edges per chunk. Returns chunk node ranges and the
    slot array (slot -> edge position in the core's sorted edge list, or -1
    for padding)."""
    nloc = NPC
    node_ptr = np.searchsorted(ld, np.arange(nloc + 1))
    deg = np.diff(node_ptr)

    bounds = [0]
    ce = cn = 0
    for n in range(nloc):
        d = deg[n]
        assert d <= SLOTS
        if ce + d > SLOTS or cn >= P:
            bounds.append(n)
            ce = cn = 0
        ce += d
        cn += 1
    bounds.append(nloc)
    nchunk = len(bounds) - 1
    if nchunk_target is not None:
        assert nchunk <= nchunk_target
        nchunk = nchunk_target

    sl = np.full((nchunk, SLOTS), -1, np.int64)   # slot -> local edge idx
    sp = np.zeros((nchunk, SLOTS), np.float32)    # slot -> node pos in chunk
    ranges = []
    for k in range(len(bounds) - 1):
        n0, n1 = bounds[k], bounds[k + 1]
        ranges.append((n0, n1))
        e0, e1 = node_ptr[n0], node_ptr[n1]
        m = e1 - e0
        assert m <= SLOTS
        sl[k, :m] = np.arange(e0, e1)
        sp[k, :m] = (ld[e0:e1] - n0).astype(np.float32)
    while len(ranges) < nchunk:
        ranges.append((nloc, nloc))
    return nchunk, ranges, sl, sp


def prepare(x, w_ij, edge_index, cutoff, Wq, Wk, Wv):
    x = np.asarray(x, np.float32)
    w_ij = np.asarray(w_ij, np.float32)
    cutoff = np.asarray(cutoff, np.float32).reshape(-1)
    src_g = np.asarray(edge_index[0], np.int64).astype(np.int32)
    dst_g = np.asarray(edge_index[1], np.int64).astype(np.int32)

    order = np.argsort(dst_g, kind="stable")
    dst_s, src_s = dst_g[order], src_g[order]
    core_lo = np.searchsorted(dst_s, np.arange(NCORE) * NPC)
    core_hi = np.searchsorted(dst_s, (np.arange(NCORE) + 1) * NPC)

    packs = []
    for c in range(NCORE):
        sl_ = slice(core_lo[c], core_hi[c])
        packs.append(_pack_core(dst_s[sl_] - c * NPC))
    nchunk_max = max(p[0] for p in packs)
    ngroup = -(-nchunk_max // G)
    nchunk = ngroup * G
    if any(p[0] != nchunk for p in packs):
        packs = []
        for c in range(NCORE):
            sl_ = slice(core_lo[c], core_hi[c])
            packs.append(_pack_core(dst_s[sl_] - c * NPC,
                                    nchunk_target=nchunk))

    w_cut = (w_ij * cutoff[:, None]).astype(NBF16)
    x_bf = x.astype(NBF16)
    wkv = np.concatenate([Wk.T, Wv.T], axis=1).astype(NBF16)
    wq = (Wq.T / np.sqrt(np.float32(HD))).astype(NBF16)
    iota = np.broadcast_to(np.arange(P, dtype=np.float32), (P, P)).astype(NBF16)

    in_maps = []
    for c in range(NCORE):
        _, ranges, sl, sp = packs[c]
        sl_l = sl.reshape(ngroup, GSLOT)
        sp_l = sp.reshape(ngroup, GSLOT)
        e0 = core_lo[c]
        # per-slot global edge ids (0 for padding; padded w rows are zero)
        eid = np.where(sl_l >= 0, order[e0 + np.maximum(sl_l, 0)], 0)
        valid = (sl_l >= 0)

        w_rows = w_cut[eid.ravel()].reshape(ngroup, GCOL, P, HID)
        w_rows[~valid.reshape(ngroup, GCOL, P)] = 0
        w_stream = w_rows.transpose(0, 2, 1, 3).copy()

        xs_rows = x_bf[src_g[eid.ravel()]].reshape(ngroup, GCOL, P, HID)
        xs_stream = xs_rows.transpose(0, 3, 1, 2).copy()
        xq_rows = x_bf[dst_g[eid.ravel()]].reshape(ngroup, GCOL, P, HID)
        xq_stream = xq_rows.transpose(0, 3, 1, 2).copy()

        pos_st = (sp_l.reshape(ngroup, GCOL, P).transpose(2, 0, 1)
                  .reshape(P, ngroup * GCOL).astype(NBF16).copy())
        in_maps.append({
            "wkv_d": wkv, "wq_d": wq, "iota_d": iota, "pos_d": pos_st,
            "we_d": w_stream, "xs_d": xs_stream, "xq_d": xq_stream,
        })
    return ngroup, in_maps, packs


def kernel(x, w_ij, edge_index, cutoff, Wq, Wk, Wv):
    ngroup, in_maps, packs = prepare(x, w_ij, edge_index, cutoff, Wq, Wk, Wv)

    if ngroup not in _cache:
        _cache[ngroup] = _build_program(ngroup)
    nc = _cache[ngroup]

    global _last_in_maps
    _last_in_maps = in_maps
    res = run_bass_kernel_spmd(nc, in_maps, core_ids=list(range(NCORE)))

    out = np.zeros((N_NODES, HID), np.float32)
    for c in range(NCORE):
        op = res.results[c]["out_d"]
        _, ranges, *_ = packs[c]
        base = c * NPC
        for k, (n0, n1) in enumerate(ranges):
            if n1 > n0:
                out[base + n0:base + n1] = op[k * P:k * P + (n1 - n0)]
    return out

